# revision 1
# baseline (speedup 1.0000x reference)
"""GPT-2 (6L, D=768, H=12, B=2, T=1024, V=50257) forward pass on 8 trn2 cores.

Sharding: tokens 2048 -> 256/core (cores 0-3 = batch 0, 4-7 = batch 1).
Attention: per-layer AllGather of K/V within each 4-core batch group; every
core computes full-kv attention for its 256 queries with an additive mask
(SPMD-uniform work). Logits: vocab-sharded (6656 padded cols/core) against
an AllGathered final hidden state; host concatenates shards.
Activations live feature-major ([d, t]) so every matmul contracts on the
partition axis with naturally-laid-out weights. Matmuls run as float32r.
"""

import sys
from contextlib import ExitStack

import numpy as np

sys.path.insert(0, "/opt/trn_rl_repo")

import concourse.bass as bass
import concourse.tile as tile
from concourse import bacc, mybir
from concourse.bass_utils import run_bass_kernel_spmd

F32 = mybir.dt.float32
F32R = mybir.dt.float32r
AF = mybir.ActivationFunctionType
ALU = mybir.AluOpType

L, D, V, B, T, H, HD = 6, 768, 50257, 2, 1024, 12, 64
NTOK = 256           # tokens per core
NC = 8               # cores
KT = D // 128        # 6 feature tiles
VSHARD = 6656        # padded vocab per core (13 * 512); 8*6656 = 53248
VT = VSHARD // 512   # 13
TT = (B * T) // 128  # 16 token tiles of the full sequence
MASKVAL = -240.0     # pre-scale additive mask (-30 after 1/8 scaling)

_CACHE = {}


def _r(x):
    return x


def build_nc(debug=False):
    nc = bacc.Bacc("TRN2", target_bir_lowering=False, debug=False, num_devices=NC)

    # ---- per-core inputs ----
    x0T = nc.dram_tensor("x0T", [D, NTOK], F32R, kind="ExternalInput")
    onesd = nc.dram_tensor("onesd", [128, 65], F32R, kind="ExternalInput")
    mask8 = nc.dram_tensor("mask8", [T, NTOK], F32, kind="ExternalInput")
    wteT = nc.dram_tensor("wteT", [KT, VT, 128, 512], F32R, kind="ExternalInput")
    # ---- replicated weights ----
    wqk = nc.dram_tensor("wqk", [L, 12, 128, KT, 128], F32R, kind="ExternalInput")
    wv = nc.dram_tensor("wv", [L, 2, 128, KT, 384], F32R, kind="ExternalInput")
    wproj = nc.dram_tensor("wproj", [L, KT, 128, KT, 128], F32R, kind="ExternalInput")
    wfc = nc.dram_tensor("wfc", [L, 24, 128, KT, 128], F32R, kind="ExternalInput")
    wfc2 = nc.dram_tensor("wfc2", [L, KT, 128, 24, 128], F32R, kind="ExternalInput")
    b_qkv = nc.dram_tensor("b_qkv", [L, 128, 12], F32, kind="ExternalInput")
    b_v = nc.dram_tensor("b_v", [L, 768], F32, kind="ExternalInput")
    b_proj = nc.dram_tensor("b_proj", [L, 128, KT], F32, kind="ExternalInput")
    b_fc = nc.dram_tensor("b_fc", [L, 128, 24], F32, kind="ExternalInput")
    b_fc2 = nc.dram_tensor("b_fc2", [L, 128, KT], F32, kind="ExternalInput")
    s_ln1 = nc.dram_tensor("s_ln1", [L, 128, KT], F32, kind="ExternalInput")
    bi_ln1 = nc.dram_tensor("bi_ln1", [L, 128, KT], F32, kind="ExternalInput")
    s_ln2 = nc.dram_tensor("s_ln2", [L, 128, KT], F32, kind="ExternalInput")
    bi_ln2 = nc.dram_tensor("bi_ln2", [L, 128, KT], F32, kind="ExternalInput")
    s_lnf = nc.dram_tensor("s_lnf", [128, KT], F32, kind="ExternalInput")
    bi_lnf = nc.dram_tensor("bi_lnf", [128, KT], F32, kind="ExternalInput")
    # ---- output ----
    out = nc.dram_tensor("out", [B * T, VSHARD], F32, kind="ExternalOutput")

    # ---- collective bounce buffers ----
    KVSZ = D * NTOK  # 196608 elems for k (and again for v)
    kv_in = nc.dram_tensor("kv_in", [2 * KVSZ], F32R)
    kv_out = nc.dram_tensor("kv_out", [8 * KVSZ], F32R)
    xf_in = nc.dram_tensor("xf_in", [KVSZ], F32R)
    xf_out = nc.dram_tensor("xf_out", [NC * KVSZ], F32R, addr_space="Shared")
    kv_groups = [[0, 1, 2, 3], [4, 5, 6, 7]]

    with tile.TileContext(nc) as tc, ExitStack() as ctx:
        const = ctx.enter_context(tc.tile_pool(name="const", bufs=1))
        ones = const.tile([128, 1], F32R)
        nc.sync.dma_start(out=ones, in_=onesd[:, 0:1])
        eps = const.tile([1, 1], F32)
        nc.vector.memset(eps, 1e-5)
        mask_sb = const.tile([128, 8, NTOK], F32)
        nc.sync.dma_start(
            out=mask_sb,
            in_=mask8.ap().rearrange("(n p) t -> p n t", p=128),
        )
        scl = const.tile([128, 4 * L + 2, KT], F32)  # ln scales/biases
        for l in range(L):
            nc.sync.dma_start(out=scl[:, 4 * l + 0, :], in_=s_ln1[l])
            nc.sync.dma_start(out=scl[:, 4 * l + 1, :], in_=bi_ln1[l])
            nc.sync.dma_start(out=scl[:, 4 * l + 2, :], in_=s_ln2[l])
            nc.sync.dma_start(out=scl[:, 4 * l + 3, :], in_=bi_ln2[l])
        nc.sync.dma_start(out=scl[:, 4 * L + 0, :], in_=s_lnf.ap())
        nc.sync.dma_start(out=scl[:, 4 * L + 1, :], in_=bi_lnf.ap())
        bias_sb = const.tile([128, L, 12 + KT + 24 + KT], F32)
        for l in range(L):
            nc.sync.dma_start(out=bias_sb[:, l, 0:12], in_=b_qkv[l])
            nc.sync.dma_start(out=bias_sb[:, l, 12 : 12 + KT], in_=b_proj[l])
            nc.sync.dma_start(out=bias_sb[:, l, 18:42], in_=b_fc[l])
            nc.sync.dma_start(out=bias_sb[:, l, 42:48], in_=b_fc2[l])

        with ExitStack() as body:
            resid = body.enter_context(tc.tile_pool(name="resid", bufs=2))
            lnp = body.enter_context(tc.tile_pool(name="lnp", bufs=1))
            qkvp = body.enter_context(tc.tile_pool(name="qkvp", bufs=1))
            kvp = body.enter_context(tc.tile_pool(name="kvp", bufs=1))
            wpool = body.enter_context(tc.tile_pool(name="wpool", bufs=1))
            w2pool = body.enter_context(tc.tile_pool(name="w2pool", bufs=2))
            att = body.enter_context(tc.tile_pool(name="att", bufs=4))
            yp = body.enter_context(tc.tile_pool(name="yp", bufs=1))
            hp = body.enter_context(tc.tile_pool(name="hp", bufs=1))
            stat = body.enter_context(tc.tile_pool(name="stat", bufs=1))
            ps_mm = body.enter_context(tc.tile_pool(name="ps_mm", bufs=2, space="PSUM"))
            ps_s = body.enter_context(tc.tile_pool(name="ps_s", bufs=2, space="PSUM"))
            ps_av = body.enter_context(tc.tile_pool(name="ps_av", bufs=2, space="PSUM"))
            ps_st = body.enter_context(tc.tile_pool(name="ps_st", bufs=1, space="PSUM"))

            dbg_n = [0]
            def dump(ap):
                if not debug:
                    return
                s = dbg_n[0]; dbg_n[0] += 1
                nc.sync.dma_start(
                    out=out.ap()[(s % 16) * 128 : (s % 16) * 128 + ap.shape[0],
                                 (s // 16) * 256 : (s // 16) * 256 + ap.free_size()]
                    .bitcast(ap.dtype),
                    in_=ap)

            x_tiles = []
            for j in range(KT):
                xt = resid.tile([128, NTOK], F32R, name=f"x{j}", tag=f"x{j}")
                nc.sync.dma_start(out=xt, in_=x0T[j * 128 : (j + 1) * 128, :])
                x_tiles.append(xt)

            def layernorm(xs, s_col, b_col):
                sums = ps_st.tile([1, NTOK], F32, name="sums", tag="sums")
                sumq = ps_st.tile([1, NTOK], F32, name="sumq", tag="sumq")
                sq = []
                for j in range(KT):
                    sqt = lnp.tile([128, NTOK], F32R, name=f"sq{j}", tag=f"sq{j}")
                    nc.vector.tensor_mul(sqt, xs[j], xs[j])
                    sq.append(sqt)
                for j in range(KT):
                    nc.tensor.matmul(sums, _r(ones), _r(xs[j]),
                                     start=(j == 0), stop=(j == KT - 1))
                for j in range(KT):
                    nc.tensor.matmul(sumq, _r(ones), _r(sq[j]),
                                     start=(j == 0), stop=(j == KT - 1))
                mu_t = stat.tile([1, NTOK], F32, name="mu_t", tag="mu_t")
                rs_t = stat.tile([1, NTOK], F32, name="rs_t", tag="rs_t")
                mu = mu_t[0:1, :]
                rs = rs_t[0:1, :]
                nc.scalar.mul(mu, sums, 1.0 / D)
                musq = stat.tile([1, NTOK], F32, name="musq", tag="musq")
                nc.vector.tensor_mul(musq, mu, mu)
                var = stat.tile([1, NTOK], F32, name="var", tag="var")
                nc.vector.tensor_scalar(out=var, in0=sumq, scalar1=1.0 / D,
                                        scalar2=None, op0=ALU.mult)
                nc.vector.tensor_sub(var, var, musq)
                nc.scalar.activation(var, var, AF.Sqrt, bias=eps)
                nc.vector.reciprocal(rs, var)
                bc = stat.tile([128, 2, NTOK], F32, name="bc", tag="bc")
                nc.gpsimd.partition_broadcast(bc[:, 0, :], mu)
                nc.gpsimd.partition_broadcast(bc[:, 1, :], rs)
                outs = []
                for j in range(KT):
                    ot = lnp.tile([128, NTOK], F32R, name=f"ln{j}", tag=f"ln{j}")
                    nc.vector.tensor_sub(ot, xs[j], bc[:, 0, :])
                    nc.vector.tensor_mul(ot, ot, bc[:, 1, :])
                    nc.vector.tensor_scalar(out=ot, in0=ot,
                                            scalar1=s_col[:, j : j + 1],
                                            scalar2=b_col[:, j : j + 1],
                                            op0=ALU.mult, op1=ALU.add)
                    outs.append(ot)
                return outs

            kv_in_ap = kv_in.ap()
            k_in = kv_in_ap[0:KVSZ].rearrange("(p t) -> p t", p=D)
            v_in = kv_in_ap[KVSZ:].rearrange("(t d) -> t d", t=NTOK)

            for l in range(L):
                sc = scl[:, 4 * l + 0, :]
                bc1 = scl[:, 4 * l + 1, :]
                ln1 = layernorm(x_tiles, sc, bc1)
                if l == 0:
                    for j in range(KT):
                        dump(x_tiles[j])
                    for j in range(KT):
                        dump(ln1[j])

                # --- QK part of QKV: feature-major out [1536, 256] ---
                qkT = []
                for ot in range(12):
                    wt = wpool.tile([128, KT, 128], F32R, name="wqk", tag="wqk", bufs=2)
                    nc.sync.dma_start(out=wt, in_=wqk[l, ot])
                    ps = ps_mm.tile([128, NTOK], F32, name="mm", tag="mm")
                    for j in range(KT):
                        nc.tensor.matmul(
                            ps, _r(wt[:, j, :]),
                            _r(ln1[j]), start=(j == 0), stop=(j == KT - 1))
                    sb = qkvp.tile([128, NTOK], F32R, name=f"qk{ot}", tag=f"qk{ot}")
                    nc.vector.tensor_scalar_add(sb, ps, bias_sb[:, l, ot : ot + 1])
                    if l == 0:
                        dump(sb)
                    qkT.append(sb)
                    if ot >= 6:  # k tiles -> collective input
                        nc.sync.dma_start(
                            out=k_in[(ot - 6) * 128 : (ot - 5) * 128, :], in_=sb)

                # --- V part: token-major out [256, 768] ---
                bv_sb = wpool.tile([128, 768], F32, name="bv", tag="bv")
                bvl = b_v.ap()[l]
                nc.sync.dma_start(
                    out=bv_sb,
                    in_=bass.AP(tensor=bvl.tensor, offset=bvl.offset,
                                ap=[[0, 128]] + list(bvl.ap)),
                )
                vloc = [qkvp.tile([128, 768], F32R, name=f"vloc{tt}", tag=f"vloc{tt}")
                        for tt in range(2)]
                for oh in range(2):
                    wt = wpool.tile([128, KT, 384], F32R, name="wvt", tag="wvt", bufs=1)
                    nc.sync.dma_start(out=wt, in_=wv[l, oh])
                    for tt in range(2):
                        ps = ps_mm.tile([128, 384], F32, name="mmv", tag="mm")
                        for j in range(KT):
                            nc.tensor.matmul(
                                ps, _r(ln1[j][:, tt * 128 : (tt + 1) * 128]),
                                _r(wt[:, j, :]),
                                start=(j == 0), stop=(j == KT - 1))
                        nc.vector.tensor_add(
                            vloc[tt][:, oh * 384 : (oh + 1) * 384], ps,
                            bv_sb[:, oh * 384 : (oh + 1) * 384])
                for tt in range(2):
                    nc.sync.dma_start(
                        out=v_in[tt * 128 : (tt + 1) * 128, :], in_=vloc[tt])
                if l == 0:
                    dump(vloc[0][:, 0:256])
                    dump(vloc[0][:, 256:512])

                nc.gpsimd.collective_compute(
                    "AllGather", ALU.bypass, replica_groups=kv_groups,
                    ins=[kv_in.ap()], outs=[kv_out.ap()[0 : 4 * 2 * KVSZ]],
                )

                # --- load gathered K (feature-major [768, 1024]) and V ---
                k_sb = [kvp.tile([128, T], F32R, name=f"k{j}", tag=f"k{j}") for j in range(KT)]
                v_sb = [kvp.tile([128, 12, 65], F32R, name=f"v{j}", tag=f"v{j}") for j in range(8)]
                kvo = kv_out.ap()
                for r in range(4):
                    base = r * 2 * KVSZ
                    k_r = kvo[base : base + KVSZ].rearrange("(p t) -> p t", p=D)
                    v_r = kvo[base + KVSZ : base + 2 * KVSZ].rearrange(
                        "(t h d) -> t h d", t=NTOK, h=12)
                    for j in range(KT):
                        nc.sync.dma_start(
                            out=k_sb[j][:, r * NTOK : (r + 1) * NTOK],
                            in_=k_r[j * 128 : (j + 1) * 128, :])
                    for tt in range(2):
                        nc.sync.dma_start(
                            out=v_sb[2 * r + tt][:, :, 0:64],
                            in_=v_r[tt * 128 : (tt + 1) * 128])
                for j in range(8):
                    nc.sync.dma_start(out=v_sb[j][:, :, 64:65], in_=onesd[:, 0:12])
                if l == 0:
                    for j in range(KT):
                        dump(k_sb[j][:, 0:256])
                    dump(v_sb[0][:, 0, :])
                    dump(v_sb[0][:, 1, :])

                # --- attention per head ---
                yT = [yp.tile([128, NTOK], F32R, name=f"y{j}", tag=f"y{j}") for j in range(KT)]
                for h in range(H):
                    p0 = 64 * (h % 2)
                    q_ap = qkT[h // 2][p0 : p0 + 64, :]
                    psy = ps_av.tile([65, NTOK], F32, name="av", tag="av")
                    for kt in range(8):
                        pss = ps_s.tile([128, NTOK], F32, name="s", tag="s")
                        nc.tensor.matmul(
                            pss,
                            _r(k_sb[h // 2][p0 : p0 + 64, kt * 128 : (kt + 1) * 128]),
                            _r(q_ap), start=True, stop=True)
                        es = att.tile([128, NTOK], F32R, name="es", tag="es")
                        nc.vector.tensor_add(es, pss, mask_sb[:, kt, :])
                        nc.scalar.activation(es, es, AF.Exp, scale=0.125)
                        if l == 0 and h < 2 and kt == 0:
                            dump(es)
                        nc.tensor.matmul(
                            psy, _r(v_sb[kt][:, h, :]), _r(es),
                            start=(kt == 0), stop=(kt == 7))
                    rec = stat.tile([1, NTOK], F32, name="rec", tag="rec")
                    nc.vector.reciprocal(rec, psy[64:65, :])
                    rb = stat.tile([64, NTOK], F32, name="rb", tag="rb")
                    nc.gpsimd.partition_broadcast(rb, rec)
                    nc.vector.tensor_mul(yT[h // 2][p0 : p0 + 64, :],
                                         psy[0:64, :], rb)
                    if l == 0 and h < 2:
                        dump(rb[0:1, :])

                # --- proj + residual ---
                x2_tiles = []
                for ot in range(KT):
                    wt = wpool.tile([128, KT, 128], F32R, name="wp", tag="wp", bufs=2)
                    nc.sync.dma_start(out=wt, in_=wproj[l, ot])
                    ps = ps_mm.tile([128, NTOK], F32, name="mm", tag="mm")
                    for j in range(KT):
                        nc.tensor.matmul(
                            ps, _r(wt[:, j, :]),
                            _r(yT[j]), start=(j == 0), stop=(j == KT - 1))
                    x2 = resid.tile([128, NTOK], F32R, name=f"x{ot}", tag=f"x{ot}")
                    nc.vector.tensor_scalar_add(ps, ps, bias_sb[:, l, 12 + ot : 13 + ot])
                    nc.vector.tensor_add(x2, ps, x_tiles[ot])
                    if l == 0:
                        dump(yT[ot])
                        dump(x2)
                    x2_tiles.append(x2)

                # --- MLP ---
                ln2 = layernorm(x2_tiles, scl[:, 4 * l + 2, :], scl[:, 4 * l + 3, :])
                h_sb = []
                for ot in range(24):
                    wt = wpool.tile([128, KT, 128], F32R, name="wf", tag="wf", bufs=2)
                    nc.sync.dma_start(out=wt, in_=wfc[l, ot])
                    ps = ps_mm.tile([128, NTOK], F32, name="mm", tag="mm")
                    for j in range(KT):
                        nc.tensor.matmul(
                            ps, _r(wt[:, j, :]),
                            _r(ln2[j]), start=(j == 0), stop=(j == KT - 1))
                    hs = hp.tile([128, NTOK], F32R, name=f"h{ot}", tag=f"h{ot}")
                    nc.scalar.activation(hs, ps, AF.Gelu_apprx_tanh,
                                         bias=bias_sb[:, l, 18 + ot : 19 + ot])
                    h_sb.append(hs)
                x3_tiles = []
                for ot in range(KT):
                    wt = w2pool.tile([128, 24, 128], F32R, name="w2", tag="w2", bufs=1)
                    nc.sync.dma_start(out=wt, in_=wfc2[l, ot])
                    ps = ps_mm.tile([128, NTOK], F32, name="mm", tag="mm")
                    for j in range(24):
                        nc.tensor.matmul(
                            ps, _r(wt[:, j, :]),
                            _r(h_sb[j]), start=(j == 0), stop=(j == 23))
                    x3 = resid.tile([128, NTOK], F32R, name=f"x{ot}", tag=f"x{ot}")
                    nc.vector.tensor_scalar_add(ps, ps, bias_sb[:, l, 42 + ot : 43 + ot])
                    nc.vector.tensor_add(x3, ps, x2_tiles[ot])
                    if l == 0:
                        dump(h_sb[4 * ot])
                        dump(x3)
                    x3_tiles.append(x3)
                x_tiles = x3_tiles

            # --- final LN + AllGather of hidden state ---
            lnf = layernorm(x_tiles, scl[:, 4 * L, :], scl[:, 4 * L + 1, :])
            xf_ap = xf_in.ap().rearrange("(p t) -> p t", p=D)
            for j in range(KT):
                nc.sync.dma_start(out=xf_ap[j * 128 : (j + 1) * 128, :], in_=lnf[j])
            nc.gpsimd.collective_compute(
                "AllGather", ALU.bypass, replica_groups=[list(range(NC))],
                ins=[xf_in.ap()], outs=[xf_out.ap()],
            )

        # --- logits: out[t, vshard] = xf.T @ wteT ---
        with ExitStack() as lg:
         if not debug:
             xfp = lg.enter_context(tc.tile_pool(name="xfp", bufs=1))
             wtep = lg.enter_context(tc.tile_pool(name="wtep", bufs=2))
             outp = lg.enter_context(tc.tile_pool(name="outp", bufs=4))
             ps_l = lg.enter_context(tc.tile_pool(name="ps_l", bufs=4, space="PSUM"))
             xfo = xf_out.ap()
             xf_sb = [xfp.tile([128, B * T], F32R, name=f"xf{j}", tag=f"xf{j}") for j in range(KT)]
             for r in range(NC):
                 x_r = xfo[r * KVSZ : (r + 1) * KVSZ].rearrange("(p t) -> p t", p=D)
                 for j in range(KT):
                     nc.sync.dma_start(
                         out=xf_sb[j][:, r * NTOK : (r + 1) * NTOK],
                         in_=x_r[j * 128 : (j + 1) * 128, :])
             for vt in range(VT):
                 wt_sb = [wtep.tile([128, 512], F32R, name=f"wte{j}", tag=f"wte{j}") for j in range(KT)]
                 for j in range(KT):
                     nc.sync.dma_start(out=wt_sb[j], in_=wteT[j, vt])
                 for tt in range(TT):
                     ps = ps_l.tile([128, 512], F32, name="lg", tag="lg")
                     for j in range(KT):
                         nc.tensor.matmul(
                             ps, _r(xf_sb[j][:, tt * 128 : (tt + 1) * 128]),
                             _r(wt_sb[j]), start=(j == 0), stop=(j == KT - 1))
                     ot = outp.tile([128, 512], F32, name="out", tag="out")
                     nc.scalar.copy(ot, ps)
                     nc.sync.dma_start(
                         out=out.ap()[tt * 128 : (tt + 1) * 128,
                                      vt * 512 : (vt + 1) * 512],
                         in_=ot)

    nc.compile()
    return nc


def prep_inputs(idx, wte, wpe, ln1_s, ln1_b, attn_w, attn_b, proj_w, proj_b,
                ln2_s, ln2_b, fc_w, fc_b, fc2_w, fc2_b, lnf_s, lnf_b):
    f = np.float32
    x0 = (wte[idx.reshape(-1)] + np.tile(wpe, (B, 1))).astype(f)  # [2048, 768]
    wte_pad = np.zeros((NC * VSHARD, D), f)
    wte_pad[:V] = wte
    shared = {
        "wqk": np.ascontiguousarray(
            attn_w[:, :, :1536].reshape(L, KT, 128, 12, 128).transpose(0, 3, 2, 1, 4)),
        "wv": np.ascontiguousarray(
            attn_w[:, :, 1536:].reshape(L, KT, 128, 2, 384).transpose(0, 3, 2, 1, 4)),
        "wproj": np.ascontiguousarray(
            proj_w.reshape(L, KT, 128, KT, 128).transpose(0, 3, 2, 1, 4)),
        "wfc": np.ascontiguousarray(
            fc_w.reshape(L, KT, 128, 24, 128).transpose(0, 3, 2, 1, 4)),
        "wfc2": np.ascontiguousarray(
            fc2_w.reshape(L, 24, 128, KT, 128).transpose(0, 3, 2, 1, 4)),
        "b_qkv": np.ascontiguousarray(
            attn_b[:, :1536].reshape(L, 12, 128).transpose(0, 2, 1)),
        "b_v": np.ascontiguousarray(attn_b[:, 1536:]),
        "b_proj": np.ascontiguousarray(proj_b.reshape(L, KT, 128).transpose(0, 2, 1)),
        "b_fc": np.ascontiguousarray(fc_b.reshape(L, 24, 128).transpose(0, 2, 1)),
        "b_fc2": np.ascontiguousarray(fc2_b.reshape(L, KT, 128).transpose(0, 2, 1)),
        "s_ln1": np.ascontiguousarray(ln1_s.reshape(L, KT, 128).transpose(0, 2, 1)),
        "bi_ln1": np.ascontiguousarray(ln1_b.reshape(L, KT, 128).transpose(0, 2, 1)),
        "s_ln2": np.ascontiguousarray(ln2_s.reshape(L, KT, 128).transpose(0, 2, 1)),
        "bi_ln2": np.ascontiguousarray(ln2_b.reshape(L, KT, 128).transpose(0, 2, 1)),
        "s_lnf": np.ascontiguousarray(lnf_s.reshape(KT, 128).T),
        "bi_lnf": np.ascontiguousarray(lnf_b.reshape(KT, 128).T),
    }
    shared = {k: v.astype(f) for k, v in shared.items()}
    in_maps = []
    tk = np.arange(T)[:, None]
    for c in range(NC):
        qs = NTOK * (c % 4)
        m = np.where(tk <= qs + np.arange(NTOK)[None, :], 0.0, MASKVAL).astype(f)
        wsh = wte_pad[c * VSHARD : (c + 1) * VSHARD]  # [6656, 768]
        wteT_t = np.ascontiguousarray(
            wsh.T.reshape(KT, 128, VT, 512).transpose(0, 2, 1, 3))
        im = dict(shared)
        im["onesd"] = np.ones((128, 65), f)
        im["x0T"] = np.ascontiguousarray(x0[c * NTOK : (c + 1) * NTOK].T)
        im["mask8"] = m
        im["wteT"] = wteT_t
        in_maps.append(im)
    return in_maps


def kernel(**inputs):
    inputs = {k: np.asarray(v) for k, v in inputs.items()}
    in_maps = prep_inputs(**inputs)
    if "nc" not in _CACHE:
        _CACHE["nc"] = build_nc()
    res = run_bass_kernel_spmd(_CACHE["nc"], in_maps, list(range(NC)))
    shards = [res.results[c]["out"] for c in range(NC)]  # each [2048, 6656]
    full = np.concatenate(shards, axis=1)[:, :V]
    return np.ascontiguousarray(full.reshape(B, T, V))



# revision 3
# speedup vs baseline: 1.0641x; 1.0641x over previous
"""GPT-2 (6L, D=768, H=12, B=2, T=1024, V=50257) forward pass on 8 trn2 cores.

v2: bf16 matmuls + f32 residual; mask-multiply on gpsimd; ln/exp-table rsqrt;
coalesced weight DMAs; host-folded LN scales and V bias; trimmed vocab pad.

Sharding: tokens 2048 -> 256/core (cores 0-3 = batch 0, 4-7 = batch 1).
Per-layer AllGather of bf16 K/V within each 4-core batch group; every core
computes full-kv attention for its 256 queries with a multiplicative 0/1 mask.
Logits: vocab-sharded (6288 cols/core, 50304 padded total) against an
AllGathered bf16 final hidden state; host concatenates shards.
"""

import sys
from contextlib import ExitStack

import numpy as np
import ml_dtypes

sys.path.insert(0, "/opt/trn_rl_repo")

import concourse.bass as bass
import concourse.tile as tile
from concourse import bacc, mybir
from concourse.bass_utils import run_bass_kernel_spmd

F32 = mybir.dt.float32
F32R = mybir.dt.float32r
BF16 = mybir.dt.bfloat16
AF = mybir.ActivationFunctionType
ALU = mybir.AluOpType
BF = ml_dtypes.bfloat16

# The act-table placement pass binds each activation func to the first listed
# table containing it.  Hide 'exp' from every table except
# 'natural_log_exp_and_others' (list order/ids unchanged) so ln and exp share
# one table: the LN rsqrt (ln,exp) and the softmax-tail (ln,exp) then never
# thrash against the softmax exp -- only gelu swaps (2 loads/layer).
import concourse.bacc as _bacc_mod

_orig_get_tables = _bacc_mod.get_activation_tables


def _exp_pinned_tables(arch):
    tabs = _orig_get_tables(arch)
    exp = mybir.ActivationFunctionType.Exp
    out = {}
    for name, funcs in tabs.items():
        if name != "natural_log_exp_and_others" and exp in funcs:
            funcs = funcs - {exp}
        out[name] = funcs
    return out


_bacc_mod.get_activation_tables = _exp_pinned_tables

L, D, V, B, T, H, HD = 6, 768, 50257, 2, 1024, 12, 64
NTOK = 256           # tokens per core
NC = 8               # cores
KT = D // 128        # 6 feature tiles
VPAD = 50304         # padded vocab (8 * 6288)
VSHARD = VPAD // NC  # 6288 = 12*512 + 144
VT = 13              # vocab tiles per core (12 full 512 + 1 of 144)
VLAST = VSHARD - 12 * 512  # 144
TT = (B * T) // 128  # 16 token tiles of the full sequence

_CACHE = {}


def build_nc():
    nc = bacc.Bacc("TRN2", target_bir_lowering=False, debug=False, num_devices=NC)

    # ---- per-core inputs ----
    x0T = nc.dram_tensor("x0T", [D, NTOK], F32R, kind="ExternalInput")
    # causal mask as a rank-1-per-column basis: scores += triA.T @ maskB adds
    # -240 to masked (key, query) entries inside the scores PSUM accumulation
    triA = nc.dram_tensor("triA", [128, 128], BF16, kind="ExternalInput")
    maskB = nc.dram_tensor("maskB", [128, 8, NTOK], BF16, kind="ExternalInput")
    wteT = nc.dram_tensor("wteT", [VT, 128, KT, 512], BF16, kind="ExternalInput")
    # ---- replicated weights (LN scales folded in host-side) ----
    wqkv = nc.dram_tensor("wqkv", [L, 128, KT, 3 * D], BF16, kind="ExternalInput")
    wproj = nc.dram_tensor("wproj", [L, 128, KT, D], BF16, kind="ExternalInput")
    wfc = nc.dram_tensor("wfc", [L, 128, KT, 4 * D], BF16, kind="ExternalInput")
    wfc2 = nc.dram_tensor("wfc2", [L, 128, 24, D], BF16, kind="ExternalInput")
    # biases: qk/proj/fc2 applied on the DVE psum-evacuation op; fc via gelu
    b_qk = nc.dram_tensor("b_qk", [L, 128, 12], F32, kind="ExternalInput")
    b_proj = nc.dram_tensor("b_proj", [L, 128, KT], F32, kind="ExternalInput")
    b_fc = nc.dram_tensor("b_fc", [L, 128, 24], F32, kind="ExternalInput")
    b_fc2 = nc.dram_tensor("b_fc2", [L, 128, KT], F32, kind="ExternalInput")
    # ---- output (bf16; host widens to f32) ----
    out = nc.dram_tensor("out", [B * T, VSHARD], BF16, kind="ExternalOutput")

    # ---- collective bounce buffers (bf16) ----
    KVSZ = D * NTOK  # 196608 elems
    kv_in = nc.dram_tensor("kv_in", [2 * KVSZ], BF16)
    kv_out = nc.dram_tensor("kv_out", [4 * 2 * KVSZ], BF16)
    xf_in = nc.dram_tensor("xf_in", [KVSZ], BF16)
    xf_out = nc.dram_tensor("xf_out", [NC * KVSZ], BF16, addr_space="Shared")
    kv_groups = [[0, 1, 2, 3], [4, 5, 6, 7]]

    with tile.TileContext(nc) as tc, ExitStack() as ctx:
        const = ctx.enter_context(tc.tile_pool(name="const", bufs=1))
        ones_f = const.tile([128, 1], F32)
        nc.vector.memset(ones_f, 1.0)
        ones_r = ones_f.bitcast(F32R)
        eps = const.tile([1, 1], F32)
        nc.vector.memset(eps, 1e-5)
        tri_sb = const.tile([128, 128], BF16)
        nc.sync.dma_start(out=tri_sb, in_=triA.ap())
        mask_sb = const.tile([128, 8, NTOK], BF16)
        nc.sync.dma_start(out=mask_sb, in_=maskB.ap())
        bqk_sb = const.tile([128, L, 12], F32)
        nc.sync.dma_start(out=bqk_sb, in_=b_qk.ap().rearrange("l p n -> p l n"))
        bfc_sb = const.tile([128, L, 24], F32)
        nc.sync.dma_start(out=bfc_sb, in_=b_fc.ap().rearrange("l p n -> p l n"))
        bpr_sb = const.tile([128, L, KT], F32)
        nc.sync.dma_start(out=bpr_sb, in_=b_proj.ap().rearrange("l p n -> p l n"))
        bf2_sb = const.tile([128, L, KT], F32)
        nc.sync.dma_start(out=bf2_sb, in_=b_fc2.ap().rearrange("l p n -> p l n"))

        # gathered K/V tiles live across the whole layer loop
        kvg = ctx.enter_context(tc.tile_pool(name="kvg", bufs=1))
        k_all = kvg.tile([128, KT, 4 * NTOK], BF16)   # feature-major keys
        v_all = kvg.tile([128, 8, H, HD + 1], BF16)   # token-major values + ones col
        nc.vector.memset(v_all[:, :, :, HD : HD + 1], 1.0)

        with ExitStack() as body:
            resid = body.enter_context(tc.tile_pool(name="resid", bufs=2))
            lnp = body.enter_context(tc.tile_pool(name="lnp", bufs=1))
            qkvp = body.enter_context(tc.tile_pool(name="qkvp", bufs=1))
            wq_p = body.enter_context(tc.tile_pool(name="wq_p", bufs=1))
            wp_p = body.enter_context(tc.tile_pool(name="wp_p", bufs=1))
            wf_p = body.enter_context(tc.tile_pool(name="wf_p", bufs=1))
            w2_p = body.enter_context(tc.tile_pool(name="w2_p", bufs=1))
            att = body.enter_context(tc.tile_pool(name="att", bufs=4))
            yp = body.enter_context(tc.tile_pool(name="yp", bufs=1))
            hp = body.enter_context(tc.tile_pool(name="hp", bufs=1))
            stat = body.enter_context(tc.tile_pool(name="stat", bufs=2))
            stat1 = body.enter_context(tc.tile_pool(name="stat1", bufs=1))
            ps_mm = body.enter_context(tc.tile_pool(name="ps_mm", bufs=3, space="PSUM"))
            ps_s = body.enter_context(tc.tile_pool(name="ps_s", bufs=2, space="PSUM"))
            ps_av = body.enter_context(tc.tile_pool(name="ps_av", bufs=2, space="PSUM"))
            ps_st = body.enter_context(tc.tile_pool(name="ps_st", bufs=1, space="PSUM"))

            x_all = resid.tile([128, KT, NTOK], F32R, name="x", tag="x")
            nc.sync.dma_start(
                out=x_all, in_=x0T.ap().rearrange("(k p) t -> p k t", p=128)
            )

            def layernorm(xs):
                """xs: [128, KT, NTOK] f32r residual -> [128, KT, NTOK] bf16."""
                sq = lnp.tile([128, KT, NTOK], F32R, name="sq", tag="sq")
                for j in range(KT):
                    nc.gpsimd.tensor_mul(sq[:, j, :], xs[:, j, :], xs[:, j, :])
                st = ps_st.tile([1, 2, NTOK], F32, name="lnst", tag="lnst")
                for j in range(KT):
                    nc.tensor.matmul(st[:, 0, :], ones_r, xs[:, j, :],
                                     start=(j == 0), stop=(j == KT - 1))
                for j in range(KT):
                    nc.tensor.matmul(st[:, 1, :], ones_r, sq[:, j, :],
                                     start=(j == 0), stop=(j == KT - 1))
                mu = stat1.tile([1, NTOK], F32, name="mu", tag="mu")
                nc.vector.tensor_scalar(out=mu, in0=st[:, 0, :], scalar1=1.0 / D,
                                        scalar2=None, op0=ALU.mult)
                var = stat1.tile([1, NTOK], F32, name="var", tag="var")
                musq = stat1.tile([1, NTOK], F32, name="musq", tag="musq")
                nc.vector.tensor_mul(musq, mu, mu)
                nc.vector.tensor_scalar(out=var, in0=st[:, 1, :], scalar1=1.0 / D,
                                        scalar2=None, op0=ALU.mult)
                nc.vector.tensor_sub(var, var, musq)
                # rs = (var+eps)^-1/2 = exp(-0.5*ln(var+eps)); stays on the
                # exp/ln ACT table (no Sqrt-table reload between softmax exps)
                lnv = stat1.tile([1, NTOK], F32, name="lnv", tag="lnv")
                nc.scalar.activation(lnv, var, AF.Ln, bias=eps)
                rs = stat1.tile([1, NTOK], F32, name="rs", tag="rs")
                nc.scalar.activation(rs, lnv, AF.Exp, scale=-0.5)
                bc = stat.tile([128, 2, NTOK], F32, name="bc", tag="bc")
                nc.gpsimd.partition_broadcast(bc[:, 0, :], mu)
                nc.gpsimd.partition_broadcast(bc[:, 1, :], rs)
                ln_o = lnp.tile([128, KT, NTOK], BF16, name="ln", tag="ln")
                for j in range(KT):
                    xm = stat.tile([128, NTOK], F32, name="xm", tag="xm")
                    nc.vector.tensor_sub(xm, xs[:, j, :], bc[:, 0, :])
                    nc.vector.tensor_mul(ln_o[:, j, :], xm, bc[:, 1, :])
                return ln_o

            kv_in_ap = kv_in.ap()
            k_in = kv_in_ap[0:KVSZ].rearrange("(k p t) -> p k t", p=128, k=KT)
            v_in = kv_in_ap[KVSZ:].rearrange("(u p d) -> p u d", p=128, u=2)
            kvo = kv_out.ap()

            for l in range(L):
                ln1 = layernorm(x_all)

                wl_qkv = wq_p.tile([128, KT, 3 * D], BF16, name="wqkv", tag="wqkv")
                nc.sync.dma_start(out=wl_qkv, in_=wqkv[l])

                # --- K part (feature-major [768, 256]) first, then V, send, AG ---
                k_loc = qkvp.tile([128, KT, NTOK], BF16, name="k_loc", tag="k_loc")
                for ot in range(6, 12):
                    ps = ps_mm.tile([128, NTOK], F32, name="mm", tag="mm")
                    for j in range(KT):
                        nc.tensor.matmul(
                            ps, wl_qkv[:, j, ot * 128 : (ot + 1) * 128],
                            ln1[:, j, :], start=(j == 0), stop=(j == KT - 1))
                    nc.vector.tensor_scalar(
                        out=k_loc[:, ot - 6, :], in0=ps,
                        scalar1=bqk_sb[:, l, ot : ot + 1], scalar2=None, op0=ALU.add)
                nc.sync.dma_start(out=k_in, in_=k_loc)

                # --- V part: token-major [2x128, 768] ---
                v_loc = qkvp.tile([128, 2, D], BF16, name="v_loc", tag="v_loc")
                for tt in range(2):
                    for oh in range(3):
                        ps = ps_mm.tile([128, NTOK], F32, name="mm", tag="mm")
                        for j in range(KT):
                            nc.tensor.matmul(
                                ps, ln1[:, j, tt * 128 : (tt + 1) * 128],
                                wl_qkv[:, j, 1536 + oh * 256 : 1536 + (oh + 1) * 256],
                                start=(j == 0), stop=(j == KT - 1))
                        nc.scalar.copy(v_loc[:, tt, oh * 256 : (oh + 1) * 256], ps)
                nc.sync.dma_start(out=v_in, in_=v_loc)

                nc.gpsimd.collective_compute(
                    "AllGather", ALU.bypass, replica_groups=kv_groups,
                    ins=[kv_in.ap()], outs=[kv_out.ap()],
                )

                # --- Q part while the AllGather is in flight ---
                q_all = qkvp.tile([128, KT, NTOK], BF16, name="q_all", tag="q_all")
                for ot in range(6):
                    ps = ps_mm.tile([128, NTOK], F32, name="mm", tag="mm")
                    for j in range(KT):
                        nc.tensor.matmul(
                            ps, wl_qkv[:, j, ot * 128 : (ot + 1) * 128],
                            ln1[:, j, :], start=(j == 0), stop=(j == KT - 1))
                    nc.vector.tensor_scalar(
                        out=q_all[:, ot, :], in0=ps,
                        scalar1=bqk_sb[:, l, ot : ot + 1], scalar2=None, op0=ALU.add)

                # prefetch next matmul weights during attention
                wl_proj = wp_p.tile([128, KT, D], BF16, name="wproj", tag="wproj")
                nc.sync.dma_start(out=wl_proj, in_=wproj[l])

                # --- load gathered K (feature-major) and V (token-major) ---
                for r in range(4):
                    base = r * 2 * KVSZ
                    k_r = kvo[base : base + KVSZ].rearrange(
                        "(k p t) -> p k t", p=128, k=KT)
                    v_r = kvo[base + KVSZ : base + 2 * KVSZ].rearrange(
                        "(u p h d) -> p u h d", p=128, u=2, h=H)
                    nc.sync.dma_start(
                        out=k_all[:, :, r * NTOK : (r + 1) * NTOK], in_=k_r)
                    for u in range(2):
                        nc.sync.dma_start(
                            out=v_all[:, 2 * r + u, :, 0:HD], in_=v_r[:, u])

                # --- attention: 12 heads x 4 pairs of key tiles.
                # Mask lands in PSUM via the triA.T@maskB matmul (start=True),
                # scores accumulate on top, exp reads PSUM directly.  Each
                # head's softmax tail is emitted one head late so the strict
                # per-engine FIFOs can't serialize head k+1's pair chain
                # behind head k's normalization.
                y_all = yp.tile([128, KT, NTOK], BF16, name="y", tag="y")

                def head_tail(h, psy):
                    p0 = 64 * (h % 2)
                    lnd = stat.tile([1, NTOK], F32, name="lnd", tag="lnd")
                    nc.scalar.activation(lnd, psy[64:65, :], AF.Ln)
                    rec = stat.tile([1, NTOK], F32, name="rec", tag="rec")
                    nc.scalar.activation(rec, lnd, AF.Exp, scale=-1.0)
                    rb = stat.tile([64, NTOK], F32, name="rb", tag="rb")
                    nc.gpsimd.partition_broadcast(rb, rec)
                    nc.vector.tensor_mul(y_all[p0 : p0 + 64, h // 2, :],
                                         psy[0:64, :], rb)

                pend = None
                for h in range(H):
                    p0 = 64 * (h % 2)
                    q_ap = q_all[p0 : p0 + 64, h // 2, :]
                    psy = ps_av.tile([65, NTOK], F32, name="av", tag="av")
                    for qd in range(4):
                        pss = ps_s.tile([128, 2, NTOK], F32, name="s", tag="s")
                        es = att.tile([128, 2, NTOK], BF16, name="es", tag="es")
                        nc.tensor.matmul(
                            pss, tri_sb, mask_sb[:, 2 * qd : 2 * qd + 2, :],
                            start=True, stop=False)
                        for ki in range(2):
                            kt = qd * 2 + ki
                            nc.tensor.matmul(
                                pss[:, ki, :],
                                k_all[p0 : p0 + 64, h // 2,
                                      kt * 128 : (kt + 1) * 128],
                                q_ap, start=False, stop=(ki == 1))
                        nc.scalar.activation(es, pss, AF.Exp, scale=0.125)
                        for ki in range(2):
                            kt = qd * 2 + ki
                            nc.tensor.matmul(
                                psy, v_all[:, kt, h, :], es[:, ki, :],
                                start=(kt == 0), stop=(kt == 7))
                        if qd == 1 and pend is not None:
                            head_tail(*pend)
                            pend = None
                    pend = (h, psy)
                head_tail(*pend)

                # --- proj + residual (bias via K=1 ones-row matmul) ---
                wl_fc = wf_p.tile([128, KT, 4 * D], BF16, name="wfc", tag="wfc")
                nc.sync.dma_start(out=wl_fc, in_=wfc[l])
                x2_all = resid.tile([128, KT, NTOK], F32R, name="x", tag="x")
                for ot in range(KT):
                    ps = ps_mm.tile([128, NTOK], F32, name="mm", tag="mm")
                    for j in range(KT):
                        nc.tensor.matmul(
                            ps, wl_proj[:, j, ot * 128 : (ot + 1) * 128],
                            y_all[:, j, :], start=(j == 0), stop=(j == KT - 1))
                    nc.vector.tensor_scalar(
                        out=ps, in0=ps, scalar1=bpr_sb[:, l, ot : ot + 1],
                        scalar2=None, op0=ALU.add)
                    nc.vector.tensor_add(x2_all[:, ot, :], ps, x_all[:, ot, :])

                # --- MLP ---
                ln2 = layernorm(x2_all)
                wl_fc2 = w2_p.tile([128, 24, D], BF16, name="wfc2", tag="wfc2")
                nc.sync.dma_start(out=wl_fc2, in_=wfc2[l])
                h_all = hp.tile([128, 24, NTOK], BF16, name="h", tag="h")
                for ot in range(24):
                    ps = ps_mm.tile([128, NTOK], F32, name="mm", tag="mm")
                    for j in range(KT):
                        nc.tensor.matmul(
                            ps, wl_fc[:, j, ot * 128 : (ot + 1) * 128],
                            ln2[:, j, :], start=(j == 0), stop=(j == KT - 1))
                    nc.scalar.activation(h_all[:, ot, :], ps, AF.Gelu_apprx_tanh,
                                         bias=bfc_sb[:, l, ot : ot + 1])
                x3_all = resid.tile([128, KT, NTOK], F32R, name="x", tag="x")
                for ot in range(KT):
                    ps = ps_mm.tile([128, NTOK], F32, name="mm", tag="mm")
                    for j in range(24):
                        nc.tensor.matmul(
                            ps, wl_fc2[:, j, ot * 128 : (ot + 1) * 128],
                            h_all[:, j, :], start=(j == 0), stop=(j == 23))
                    nc.vector.tensor_scalar(
                        out=ps, in0=ps, scalar1=bf2_sb[:, l, ot : ot + 1],
                        scalar2=None, op0=ALU.add)
                    nc.vector.tensor_add(x3_all[:, ot, :], ps, x2_all[:, ot, :])
                x_all = x3_all

            # --- final LN + AllGather of hidden state (bf16) ---
            lnf = layernorm(x_all)
            nc.sync.dma_start(
                out=xf_in.ap().rearrange("(k p t) -> p k t", p=128, k=KT), in_=lnf)
            nc.gpsimd.collective_compute(
                "AllGather", ALU.bypass, replica_groups=[list(range(NC))],
                ins=[xf_in.ap()], outs=[xf_out.ap()],
            )

        # --- logits: out[t, vshard] = xf.T @ wteT ---
        with ExitStack() as lg:
            xfp = lg.enter_context(tc.tile_pool(name="xfp", bufs=1))
            wtep = lg.enter_context(tc.tile_pool(name="wtep", bufs=2))
            outp = lg.enter_context(tc.tile_pool(name="outp", bufs=3))
            ps_l = lg.enter_context(tc.tile_pool(name="ps_l", bufs=4, space="PSUM"))
            xfo = xf_out.ap()
            xf_sb = xfp.tile([128, KT, B * T], BF16, name="xf", tag="xf")
            for r in range(NC):
                x_r = xfo[r * KVSZ : (r + 1) * KVSZ].rearrange(
                    "(k p t) -> p k t", p=128, k=KT)
                nc.sync.dma_start(
                    out=xf_sb[:, :, r * NTOK : (r + 1) * NTOK], in_=x_r)
            for vt in range(VT):
                vw = 512 if vt < 12 else VLAST
                wt_sb = wtep.tile([128, KT, 512], BF16, name="wte", tag="wte")
                nc.sync.dma_start(out=wt_sb, in_=wteT[vt])
                for tg in range(4):  # groups of 4 token tiles
                    ot = outp.tile([128, 4, 512], BF16, name="out", tag="out")
                    for ti in range(4):
                        tt = tg * 4 + ti
                        ps = ps_l.tile([128, 512], F32, name="lg", tag="lg")
                        for j in range(KT):
                            nc.tensor.matmul(
                                ps[:, 0:vw],
                                xf_sb[:, j, tt * 128 : (tt + 1) * 128],
                                wt_sb[:, j, 0:vw],
                                start=(j == 0), stop=(j == KT - 1))
                        if ti % 2 == 0:
                            nc.scalar.copy(ot[:, ti, 0:vw], ps[:, 0:vw])
                        else:
                            nc.vector.tensor_copy(ot[:, ti, 0:vw], ps[:, 0:vw])
                    nc.sync.dma_start(
                        out=out.ap()[tg * 512 : (tg + 1) * 512,
                                     vt * 512 : vt * 512 + vw].rearrange(
                                         "(f p) v -> p f v", p=128),
                        in_=ot[:, :, 0:vw])

    nc.compile()
    return nc


def prep_inputs(idx, wte, wpe, ln1_s, ln1_b, attn_w, attn_b, proj_w, proj_b,
                ln2_s, ln2_b, fc_w, fc_b, fc2_w, fc2_b, lnf_s, lnf_b):
    f = np.float32
    idx = np.asarray(idx)
    x0 = (wte[idx.reshape(-1)] + np.tile(wpe, (B, 1))).astype(f)  # [2048, 768]

    # fold LN1 scale into qkv weights, LN1 bias into qkv bias
    wqkv_f = ln1_s[:, :, None] * attn_w                     # [L, 768, 2304]
    bqkv_f = np.einsum("ld,ldo->lo", ln1_b, attn_w) + attn_b
    # fold V bias through proj (softmax rows sum to 1)
    bv = bqkv_f[:, 1536:]                                   # [L, 768]
    bproj_f = np.einsum("ld,ldo->lo", bv, proj_w) + proj_b  # [L, 768]
    # fold LN2 scale/bias into fc
    wfc_f = ln2_s[:, :, None] * fc_w
    bfc_f = np.einsum("ld,ldo->lo", ln2_b, fc_w) + fc_b
    # fold LNf scale into wte head; LNf bias becomes a host-side logit offset
    wte_h = (wte * lnf_s[None, :]).astype(f)
    logit_off = (wte @ lnf_b).astype(f)                     # [V]

    wte_pad = np.zeros((VPAD, D), f)
    wte_pad[:V] = wte_h

    shared = {
        "wqkv": np.ascontiguousarray(
            wqkv_f.reshape(L, KT, 128, 3 * D).transpose(0, 2, 1, 3)).astype(BF),
        "wproj": np.ascontiguousarray(
            proj_w.reshape(L, KT, 128, D).transpose(0, 2, 1, 3)).astype(BF),
        "wfc": np.ascontiguousarray(
            wfc_f.reshape(L, KT, 128, 4 * D).transpose(0, 2, 1, 3)).astype(BF),
        "wfc2": np.ascontiguousarray(
            fc2_w.reshape(L, 24, 128, D).transpose(0, 2, 1, 3)).astype(BF),
        "b_qk": np.ascontiguousarray(
            bqkv_f[:, :1536].reshape(L, 12, 128).transpose(0, 2, 1)).astype(f),
        "b_proj": np.ascontiguousarray(
            bproj_f.reshape(L, KT, 128).transpose(0, 2, 1)).astype(f),
        "b_fc": np.ascontiguousarray(
            bfc_f.reshape(L, 24, 128).transpose(0, 2, 1)).astype(f),
        "b_fc2": np.ascontiguousarray(
            fc2_b.reshape(L, KT, 128).transpose(0, 2, 1)).astype(f),
        "triA": np.triu(np.full((128, 128), -240.0, f)).astype(BF),
    }
    in_maps = []
    for c in range(NC):
        qs = NTOK * (c % 4)
        # maskB[j0, kt, q] = 1 adds -240 to keys k >= j0 of tile kt for query q
        mB = np.zeros((128, 8, NTOK), f)
        qcol = np.arange(NTOK)
        for kt in range(8):
            j0 = np.clip(qs + qcol - kt * 128 + 1, 0, 128)
            sel = j0 <= 127
            mB[j0[sel], kt, qcol[sel]] = 1.0
        wsh = wte_pad[c * VSHARD : (c + 1) * VSHARD]          # [6288, 768]
        wtile = np.zeros((VT, 512, D), f)
        wtile[:12] = wsh[: 12 * 512].reshape(12, 512, D)
        wtile[12, :VLAST] = wsh[12 * 512 :]
        im = dict(shared)
        im["x0T"] = np.ascontiguousarray(x0[c * NTOK : (c + 1) * NTOK].T).astype(f)
        im["maskB"] = mB.astype(BF)
        im["wteT"] = np.ascontiguousarray(
            wtile.reshape(VT, 512, KT, 128).transpose(0, 3, 2, 1)).astype(BF)
        in_maps.append(im)
    return in_maps, logit_off


def kernel(**inputs):
    inputs = {k: np.asarray(v) for k, v in inputs.items()}
    in_maps, logit_off = prep_inputs(**inputs)
    if "nc" not in _CACHE:
        _CACHE["nc"] = build_nc()
    res = run_bass_kernel_spmd(_CACHE["nc"], in_maps, list(range(NC)))
    shards = [np.asarray(res.results[c]["out"]).astype(np.float32)
              for c in range(NC)]  # each [2048, 6288] bf16 -> f32
    full = np.concatenate(shards, axis=1)[:, :V]
    full = full + logit_off[None, :V]
    return np.ascontiguousarray(full.reshape(B, T, V))
